# revision 23
# baseline (speedup 1.0000x reference)
"""Chamfer distance kernel for 8x Trainium2 NeuronCores (Bass/Tile).

Problem: xyz1 [2,8192,3] f32, xyz2 [2,8192,3] f32 ->
  dist1 [2,8192] f32, dist2 [2,8192] f32, idx1 [2,8192] i32, idx2 [2,8192] i32
  (squared L2 nearest-neighbor distances + argmins, both directions).

Sharding: core c owns rows c*1024:(c+1)*1024 of xyz1 (forward direction,
min over full xyz2) AND rows c*1024:(c+1)*1024 of xyz2 (reverse direction,
min over full xyz1). Each core's outputs are exact output shards -> host
just concatenates (no collectives).

Math: the device computes, per query q and db point x_j,
  e[q,j] = 2 q.x_j - |x_j|^2 - |q|^2  (= -|q - x_j|^2, so argmax_j e =
argmin_j d and |e| is SMALL near the max -> fp16 keeps ~2^-11 relative
precision exactly where the argmax is decided). e is produced by ONE bf16
matmul with K=24 packed rows: each fp32 operand split into bf16 limbs
(h/m/l), all cross terms down to ~2^-27 relative; -|x|^2 and -|q|^2 are
folded in via constant-one rows.

Per 128-query row-tile the device then:
  - 16 matmuls [128,512] into a manually-scheduled [128,4096] PSUM region
    (8 groups of 1024 cols, two rounds over the region; slice-level tile
    deps give a 4-deep rotation)
  - evacuates PSUM: ScalarE copies 6.5 groups to SBUF as fp16 (three
    2048-wide copies + one 512) while the DVE max-folds the remaining 1.5
    groups directly from PSUM (evacuation + first fold level in one pass).
    GPSIMD cannot help: the Pool engine has no max ALU op on TRN2.
  - binary max-fold tree (pairs chosen within PSUM rounds so every copy's
    source is written ahead of ScalarE's queue) down to f5 [128,128],
    all on the DVE at fp16 2x_1P
  - InstMax + InstMaxIndex on f5 give the argmax SLOT (8 deep)
The host expands slot -> 64 column candidates (stride 128), evaluates the
true distances in fp64, and takes the min: exact dist at the chosen
index, index correct up to fp16 near-ties (same class of ties the fp32
reference resolves arbitrarily itself).
"""

import numpy as np
import ml_dtypes

import concourse.bacc as bacc
import concourse.mybir as mybir
import concourse.tile as tile
from concourse.bass_utils import run_bass_kernel_spmd

BF16 = ml_dtypes.bfloat16
F32 = np.float32

NCORES = 8
B, N, M, C = 2, 8192, 8192, 3
SLAB = N // NCORES            # 1024 queries per core per problem
NPROB = 2 * B                 # (fwd,b0),(fwd,b1),(rev,b0),(rev,b1)
K = 24                        # packed contraction rows
TQ = 128                      # queries per row-tile (partitions)
NT = SLAB // TQ               # 8 row-tiles per problem
MCH = 512                     # matmul free chunk (one PSUM bank)
PSW = 2048                    # psum tile width (4 banks; bufs=2 => 8 banks)
NG = M // PSW                 # 4 psum groups per row-tile
NSLOT = 128                   # folded slot count (f6 width)
NCAND = M // NSLOT            # 64 candidates per slot
PROBW = SLAB + M              # per-problem packed operand width


def _split3(x):
    """fp32 -> three bf16 limbs (as fp32 arrays) with x ~= h+m+l to ~2^-27."""
    x = x.astype(F32)
    h = x.astype(BF16)
    r = (x - h.astype(F32)).astype(F32)
    m = r.astype(BF16)
    r2 = (r - m.astype(F32)).astype(F32)
    l = r2.astype(BF16)
    return h.astype(F32), m.astype(F32), l.astype(F32)


def _pack_terms(q2, db, neg_sqdb, neg_sqq):
    """Build the K=24 (lhs_col, rhs_row) packed operands.

    q2: [Nq,3] fp32 (queries, ALREADY scaled by 2)
    db: [M,3] fp32, neg_sqdb: [M] fp32 (= -|db|^2), neg_sqq: [Nq] (= -|q|^2)
    Returns lhsT [K,Nq] bf16, rhs [K,M] bf16. Big terms first so PSUM
    partial sums stay well-scaled.
    """
    h1, m1, l1 = _split3(q2)
    h2, m2, l2 = _split3(db)
    sh, sm, sl = _split3(neg_sqdb)
    qh, qm, ql = _split3(neg_sqq)
    ones_n = np.ones(q2.shape[0], F32)
    ones_m = np.ones(db.shape[0], F32)
    lhs_rows, rhs_rows = [], []

    def add(lc, rr):
        lhs_rows.append(lc)
        rhs_rows.append(rr)

    for c in range(3):
        add(h1[:, c], h2[:, c])
    add(ones_n, sh)
    add(qh, ones_m)
    for c in range(3):
        add(h1[:, c], m2[:, c])
        add(m1[:, c], h2[:, c])
    add(ones_n, sm)
    add(qm, ones_m)
    for c in range(3):
        add(m1[:, c], m2[:, c])
        add(h1[:, c], l2[:, c])
        add(l1[:, c], h2[:, c])
    add(ones_n, sl)
    add(ql, ones_m)
    assert len(lhs_rows) == K
    lhsT = np.stack(lhs_rows, 0).astype(BF16)
    rhs = np.stack(rhs_rows, 0).astype(BF16)
    return lhsT, rhs


def _build_nc():
    # Bacc (not plain Bass): its compile() pipeline moves matmul waits onto
    # ldweights and splits multi-wait instructions via event semaphores.
    nc = bacc.Bacc("TRN2", target_bir_lowering=False, debug=False)
    comb_d = nc.dram_tensor("comb", [K, NPROB * PROBW], mybir.dt.bfloat16,
                            kind="ExternalInput")
    # argmax slot per (problem, tile): 8 MaxIndex outputs, col 0 is the slot.
    outi_d = nc.dram_tensor("outi", [TQ, NPROB * NT * 8], mybir.dt.uint16,
                            kind="ExternalOutput")

    F16 = mybir.dt.float16
    MAX = mybir.AluOpType.max

    with tile.TileContext(nc) as tc:
        with (
            tc.tile_pool(name="const", bufs=1) as constp,
            tc.tile_pool(name="ebuf", bufs=3) as ep,
            tc.tile_pool(name="psum", bufs=1, space="PSUM") as pp,
        ):
            comb_t = constp.tile([K, NPROB * PROBW], mybir.dt.bfloat16)
            # DMA order follows first-use order inside problem 0 (queries,
            # then rhs groups g4g5, g6g7, g3, g1, g2, g0) so compute starts
            # asap; remaining problems stream behind compute.
            for a, b_ in ((0, SLAB), (SLAB + 4096, SLAB + 6144),
                          (SLAB + 6144, PROBW), (SLAB + 3072, SLAB + 4096),
                          (SLAB + 1024, SLAB + 2048),
                          (SLAB + 2048, SLAB + 3072), (SLAB, SLAB + 1024)):
                nc.sync.dma_start(comb_t[:, a:b_], comb_d[:, a:b_])
            for p in range(1, NPROB):
                sl_p = slice(p * PROBW, (p + 1) * PROBW)
                nc.sync.dma_start(comb_t[:, sl_p], comb_d[:, sl_p])
            outi_t = constp.tile([TQ, NPROB * NT * 8], mybir.dt.uint16)
            # hoist the ScalarE activation-table load (~2.7us) into the
            # initial DMA window: prime it with a tiny copy on scratch.
            atl_t = constp.tile([TQ, 8], mybir.dt.float32)
            nc.vector.memset(atl_t[:], 0.0)
            nc.scalar.copy(atl_t[:], atl_t[:])

            # one manually-scheduled PSUM region [TQ, 4096] (all 8 banks);
            # slice-level dep tracking gives 4-deep 1024-col rotation while
            # letting ScalarE evacuate 2048-col spans in one instruction.
            P = pp.tile([TQ, 4096], mybir.dt.float32)

            for p in range(NPROB):
                base = p * PROBW
                for t in range(NT):
                    lhs = comb_t[:, base + t * TQ:base + (t + 1) * TQ]
                    # s_t: [s4 | s5 | s6 | s7 | s3 | s1 | s0lo(512)] (fp16)
                    s_t = ep.tile([TQ, 6 * 1024 + 512], F16, tag="s")
                    q1 = ep.tile([TQ, 1024], F16, tag="q1")
                    q2 = ep.tile([TQ, 1024], F16, tag="q2")
                    v1 = ep.tile([TQ, 1024], F16, tag="v1")
                    v2 = ep.tile([TQ, 1024], F16, tag="v2")
                    r1 = ep.tile([TQ, 1024], F16, tag="r1")
                    r2 = ep.tile([TQ, 1024], F16, tag="r2")
                    ff = ep.tile([TQ, 1024], F16, tag="ff")
                    f3 = ep.tile([TQ, 512], F16, tag="f3")
                    f4 = ep.tile([TQ, 256], F16, tag="f4")
                    f5 = ep.tile([TQ, NSLOT], F16, tag="f5")
                    m8 = ep.tile([TQ, 8], F16, tag="m8")

                    def mm(g, pcol):
                        # group g (db cols [g*1024,(g+1)*1024)) -> P[:, pcol:+1024]
                        o = base + SLAB + g * 1024
                        for j in range(1024 // MCH):
                            nc.tensor.matmul(
                                P[:, pcol + j * MCH:pcol + (j + 1) * MCH],
                                lhs,
                                comb_t[:, o + j * MCH:o + (j + 1) * MCH],
                                start=True, stop=True,
                            )

                    # 1024-col groups g0..g7 paired within PSUM rounds:
                    # q1=max(g4,g5), q2=max(g6,g7), v1=max(g2,g3),
                    # v2=max(g0,g1); the final 128 slots still cover
                    # {j + 128k}. ScalarE evacuates g4..g7 + g3,g1 + g0lo
                    # (3.5 copies); the DVE folds g2, g0hi from PSUM.
                    # Emission order keeps each copy's source one copy-
                    # duration ahead of ScalarE's queue.
                    mm(4, 0)
                    mm(5, 1024)
                    nc.scalar.copy(s_t[:, 0:2048], P[:, 0:2048])
                    mm(6, 2048)
                    mm(7, 3072)
                    nc.scalar.copy(s_t[:, 2048:4096], P[:, 2048:4096])
                    nc.vector.tensor_tensor(                 # q1 = (s4,s5)
                        q1[:], s_t[:, 0:1024], s_t[:, 1024:2048], op=MAX)
                    nc.vector.tensor_tensor(                 # q2 = (s6,s7)
                        q2[:], s_t[:, 2048:3072], s_t[:, 3072:4096], op=MAX)
                    mm(3, 0)
                    mm(1, 1024)
                    # [s3 | s1] in one evacuation
                    nc.scalar.copy(s_t[:, 4096:6144], P[:, 0:2048])
                    nc.vector.tensor_tensor(r1[:], q1[:], q2[:], op=MAX)
                    mm(2, 2048)
                    nc.vector.tensor_tensor(                 # v1 = (g2, s3)
                        v1[:], P[:, 2048:3072], s_t[:, 4096:5120], op=MAX)
                    mm(0, 3072)
                    nc.scalar.copy(s_t[:, 6144:6656], P[:, 3072:3584])
                    nc.vector.tensor_tensor(                 # v2hi = (g0hi, s1hi)
                        v2[:, 512:1024], P[:, 3584:4096],
                        s_t[:, 5632:6144], op=MAX)
                    nc.vector.tensor_tensor(                 # v2lo = (s0lo, s1lo)
                        v2[:, 0:512], s_t[:, 6144:6656],
                        s_t[:, 5120:5632], op=MAX)
                    nc.vector.tensor_tensor(r2[:], v1[:], v2[:], op=MAX)
                    nc.vector.tensor_tensor(ff[:], r1[:], r2[:], op=MAX)
                    nc.vector.tensor_tensor(
                        f3[:], ff[:, 0:512], ff[:, 512:1024], op=MAX)
                    nc.vector.tensor_tensor(
                        f4[:], f3[:, 0:256], f3[:, 256:512], op=MAX)
                    nc.vector.tensor_tensor(
                        f5[:], f4[:, 0:128], f4[:, 128:256], op=MAX)
                    ob = (p * NT + t) * 8
                    nc.vector.max(m8[:], f5[:])
                    nc.vector.max_index(outi_t[:, ob:ob + 8], m8[:], f5[:])
            nc.sync.dma_start(outi_d[:], outi_t[:])
    nc.compile()
    return nc


_NC = None
LAST_RESULTS = None  # most recent BassKernelResults (for profiling harnesses)


def _get_nc():
    global _NC
    if _NC is None:
        _NC = _build_nc()
    return _NC


def _prep_inputs(xyz1, xyz2):
    """Build per-core in_maps."""
    xyz1 = np.asarray(xyz1, F32)
    xyz2 = np.asarray(xyz2, F32)
    sq1 = (xyz1.astype(np.float64) ** 2).sum(-1).astype(F32)  # [B,N]
    sq2 = (xyz2.astype(np.float64) ** 2).sum(-1).astype(F32)  # [B,M]

    combs = [np.empty((K, NPROB * PROBW), BF16) for _ in range(NCORES)]
    for b in range(B):
        for rev in (0, 1):
            p = 2 * rev + b
            qsrc = xyz2[b] if rev else xyz1[b]
            dbsrc = xyz1[b] if rev else xyz2[b]
            nsqdb = -(sq1[b] if rev else sq2[b])
            nsqq = -(sq2[b] if rev else sq1[b])
            lhsT_full, rhs = _pack_terms((2.0 * qsrc).astype(F32), dbsrc,
                                         nsqdb, nsqq)
            for c in range(NCORES):
                sl = slice(p * PROBW, p * PROBW + SLAB)
                combs[c][:, sl] = lhsT_full[:, c * SLAB:(c + 1) * SLAB]
                combs[c][:, p * PROBW + SLAB:(p + 1) * PROBW] = rhs
    return [{"comb": combs[c]} for c in range(NCORES)]


def kernel(xyz1, xyz2):
    xyz1 = np.asarray(xyz1, F32)
    xyz2 = np.asarray(xyz2, F32)
    nc = _get_nc()
    in_maps = _prep_inputs(xyz1, xyz2)
    global LAST_RESULTS
    LAST_RESULTS = run_bass_kernel_spmd(nc, in_maps, list(range(NCORES)))
    res = LAST_RESULTS.results

    # assemble argmax slots: slots[p][global query row] in [0, 256)
    slots = np.empty((NPROB, N), np.int64)
    for c in range(NCORES):
        outi = np.asarray(res[c]["outi"]).reshape(TQ, NPROB * NT * 8)
        for p in range(NPROB):
            for t in range(NT):
                rows = slice(c * SLAB + t * TQ, c * SLAB + (t + 1) * TQ)
                slots[p, rows] = outi[:, (p * NT + t) * 8].astype(np.int64)

    dist1 = np.empty((B, N), F32)
    dist2 = np.empty((B, M), F32)
    idx1 = np.empty((B, N), np.int32)
    idx2 = np.empty((B, M), np.int32)
    offs = NSLOT * np.arange(NCAND)[None, :]        # [1, 32]
    for b in range(B):
        for rev in (0, 1):
            p = 2 * rev + b
            q = (xyz2[b] if rev else xyz1[b]).astype(np.float64)
            db = (xyz1[b] if rev else xyz2[b]).astype(np.float64)
            cand = slots[p][:, None] + offs                  # [N, 32]
            dd = ((q[:, None, :] - db[cand]) ** 2).sum(-1)   # [N, 32] fp64
            best = dd.argmin(1)
            rows = np.arange(N)
            if rev:
                dist2[b] = dd[rows, best].astype(F32)
                idx2[b] = cand[rows, best].astype(np.int32)
            else:
                dist1[b] = dd[rows, best].astype(F32)
                idx1[b] = cand[rows, best].astype(np.int32)
    return dist1, dist2, idx1, idx2


# revision 30
# speedup vs baseline: 1.0372x; 1.0372x over previous
"""Chamfer distance kernel for 8x Trainium2 NeuronCores (Bass/Tile).

Problem: xyz1 [2,8192,3] f32, xyz2 [2,8192,3] f32 ->
  dist1 [2,8192] f32, dist2 [2,8192] f32, idx1 [2,8192] i32, idx2 [2,8192] i32
  (squared L2 nearest-neighbor distances + argmins, both directions).

Sharding: core c owns rows c*1024:(c+1)*1024 of xyz1 (forward direction,
min over full xyz2) AND rows c*1024:(c+1)*1024 of xyz2 (reverse direction,
min over full xyz1). Each core's outputs are exact output shards -> host
just concatenates (no collectives).

Math: the device computes, per query q and db point x_j,
  e[q,j] = 2 q.x_j - |x_j|^2 - |q|^2  (= -|q - x_j|^2, so argmax_j e =
argmin_j d and |e| is SMALL near the max -> fp16 keeps ~2^-11 relative
precision exactly where the argmax is decided). e is produced by ONE bf16
matmul with K=24 packed rows: each fp32 operand split into bf16 limbs
(h/m/l), all cross terms down to ~2^-27 relative; -|x|^2 and -|q|^2 are
folded in via constant-one rows.

Per 128-query row-tile the device then:
  - 16 matmuls [128,512] into a manually-scheduled [128,4096] PSUM region
    (8 groups of 1024 cols, two rounds over the region; slice-level tile
    deps give a 4-deep rotation)
  - evacuates PSUM: ScalarE copies 6.5 groups to SBUF as fp16 (three
    2048-wide copies + one 512) while the DVE max-folds the remaining 1.5
    groups directly from PSUM (evacuation + first fold level in one pass).
    GPSIMD cannot help: the Pool engine has no max ALU op on TRN2.
  - binary max-fold tree (pairs chosen within PSUM rounds so every copy's
    source is written ahead of ScalarE's queue) down to f5 [128,128],
    all on the DVE at fp16 2x_1P
  - InstMax + InstMaxIndex on f5 give the argmax SLOT (8 deep)
The host expands slot -> 64 column candidates (stride 128), evaluates the
true distances in fp64, and takes the min: exact dist at the chosen
index, index correct up to fp16 near-ties (same class of ties the fp32
reference resolves arbitrarily itself).
"""

import numpy as np
import ml_dtypes

import concourse.bacc as bacc
import concourse.mybir as mybir
import concourse.tile as tile
from concourse.bass_utils import run_bass_kernel_spmd

BF16 = ml_dtypes.bfloat16
F32 = np.float32

NCORES = 8
B, N, M, C = 2, 8192, 8192, 3
SLAB = N // NCORES            # 1024 queries per core per problem
NPROB = 2 * B                 # (fwd,b0),(fwd,b1),(rev,b0),(rev,b1)
K = 24                        # packed contraction rows
TQ = 128                      # queries per row-tile (partitions)
NT = SLAB // TQ               # 8 row-tiles per problem
MCH = 512                     # matmul free chunk (one PSUM bank)
PSW = 2048                    # psum tile width (4 banks; bufs=2 => 8 banks)
NG = M // PSW                 # 4 psum groups per row-tile
NSLOT = 128                   # folded slot count (f6 width)
NCAND = M // NSLOT            # 64 candidates per slot
PROBW = SLAB + M              # per-problem packed operand width


def _split3(x):
    """fp32 -> three bf16 limbs (as fp32 arrays) with x ~= h+m+l to ~2^-27."""
    x = x.astype(F32)
    h = x.astype(BF16)
    r = (x - h.astype(F32)).astype(F32)
    m = r.astype(BF16)
    r2 = (r - m.astype(F32)).astype(F32)
    l = r2.astype(BF16)
    return h.astype(F32), m.astype(F32), l.astype(F32)


def _pack_terms(q2, db, neg_sqdb, neg_sqq):
    """Build the K=24 (lhs_col, rhs_row) packed operands.

    q2: [Nq,3] fp32 (queries, ALREADY scaled by 2)
    db: [M,3] fp32, neg_sqdb: [M] fp32 (= -|db|^2), neg_sqq: [Nq] (= -|q|^2)
    Returns lhsT [K,Nq] bf16, rhs [K,M] bf16. Big terms first so PSUM
    partial sums stay well-scaled.
    """
    h1, m1, l1 = _split3(q2)
    h2, m2, l2 = _split3(db)
    sh, sm, sl = _split3(neg_sqdb)
    qh, qm, ql = _split3(neg_sqq)
    ones_n = np.ones(q2.shape[0], F32)
    ones_m = np.ones(db.shape[0], F32)
    lhs_rows, rhs_rows = [], []

    def add(lc, rr):
        lhs_rows.append(lc)
        rhs_rows.append(rr)

    for c in range(3):
        add(h1[:, c], h2[:, c])
    add(ones_n, sh)
    add(qh, ones_m)
    for c in range(3):
        add(h1[:, c], m2[:, c])
        add(m1[:, c], h2[:, c])
    add(ones_n, sm)
    add(qm, ones_m)
    for c in range(3):
        add(m1[:, c], m2[:, c])
        add(h1[:, c], l2[:, c])
        add(l1[:, c], h2[:, c])
    add(ones_n, sl)
    add(ql, ones_m)
    assert len(lhs_rows) == K
    lhsT = np.stack(lhs_rows, 0).astype(BF16)
    rhs = np.stack(rhs_rows, 0).astype(BF16)
    return lhsT, rhs


def _build_nc():
    # Bacc (not plain Bass): its compile() pipeline moves matmul waits onto
    # ldweights and splits multi-wait instructions via event semaphores.
    nc = bacc.Bacc("TRN2", target_bir_lowering=False, debug=False)
    comb_d = nc.dram_tensor("comb", [K, NPROB * PROBW], mybir.dt.bfloat16,
                            kind="ExternalInput")
    # argmax slot per (problem, tile): 8 MaxIndex outputs, col 0 is the slot.
    outi_d = nc.dram_tensor("outi", [TQ, NPROB * NT * 8], mybir.dt.uint16,
                            kind="ExternalOutput")

    F16 = mybir.dt.float16
    MAX = mybir.AluOpType.max

    with tile.TileContext(nc) as tc:
        with (
            tc.tile_pool(name="const", bufs=1) as constp,
            tc.tile_pool(name="ebuf", bufs=3) as ep,
            tc.tile_pool(name="psum", bufs=1, space="PSUM") as pp,
        ):
            comb_t = constp.tile([K, NPROB * PROBW], mybir.dt.bfloat16)
            # DMA order follows first-use order inside problem 0 (queries,
            # then rhs groups g4g5, g6g7, g3, g1, g2, g0) so compute starts
            # asap; remaining problems stream behind compute.
            for a, b_ in ((0, SLAB), (SLAB + 4096, SLAB + 5120),
                          (SLAB + 5120, SLAB + 6144),
                          (SLAB + 6144, PROBW), (SLAB + 3072, SLAB + 4096),
                          (SLAB + 1024, SLAB + 2048),
                          (SLAB + 2048, SLAB + 3072), (SLAB, SLAB + 1024)):
                nc.sync.dma_start(comb_t[:, a:b_], comb_d[:, a:b_])
            for p in range(1, NPROB):
                sl_p = slice(p * PROBW, (p + 1) * PROBW)
                nc.sync.dma_start(comb_t[:, sl_p], comb_d[:, sl_p])
            outi_t = constp.tile([TQ, NPROB * NT * 8], mybir.dt.uint16)
            # hoist the ScalarE activation-table load (~2.7us) into the
            # initial DMA window: prime it with a tiny copy on scratch.
            atl_t = constp.tile([TQ, 8], mybir.dt.float32)
            nc.vector.memset(atl_t[:], 0.0)
            nc.scalar.copy(atl_t[:], atl_t[:])

            # one manually-scheduled PSUM region [TQ, 4096] (all 8 banks);
            # slice-level dep tracking gives 4-deep 1024-col rotation while
            # letting ScalarE evacuate 2048-col spans in one instruction.
            P = pp.tile([TQ, 4096], mybir.dt.float32)

            for p in range(NPROB):
                base = p * PROBW
                for t in range(NT):
                    lhs = comb_t[:, base + t * TQ:base + (t + 1) * TQ]
                    # s_t: [s4 | s5 | s6 | s7 | s3 | s1 | s0lo(512)] (fp16)
                    s_t = ep.tile([TQ, 6 * 1024 + 512], F16, tag="s")
                    q1 = ep.tile([TQ, 1024], F16, tag="q1")
                    q2 = ep.tile([TQ, 1024], F16, tag="q2")
                    v1 = ep.tile([TQ, 1024], F16, tag="v1")
                    v2 = ep.tile([TQ, 1024], F16, tag="v2")
                    r1 = ep.tile([TQ, 1024], F16, tag="r1")
                    r2 = ep.tile([TQ, 1024], F16, tag="r2")
                    ff = ep.tile([TQ, 1024], F16, tag="ff")
                    f3 = ep.tile([TQ, 512], F16, tag="f3")
                    f4 = ep.tile([TQ, 256], F16, tag="f4")
                    f5 = ep.tile([TQ, NSLOT], F16, tag="f5")
                    m8 = ep.tile([TQ, 8], F16, tag="m8")

                    def mm(g, pcol):
                        # group g (db cols [g*1024,(g+1)*1024)) -> P[:, pcol:+1024]
                        o = base + SLAB + g * 1024
                        for j in range(1024 // MCH):
                            nc.tensor.matmul(
                                P[:, pcol + j * MCH:pcol + (j + 1) * MCH],
                                lhs,
                                comb_t[:, o + j * MCH:o + (j + 1) * MCH],
                                start=True, stop=True,
                            )

                    # 1024-col groups g0..g7 paired within PSUM rounds:
                    # q1=max(g4,g5), q2=max(g6,g7), v1=max(g2,g3),
                    # v2=max(g0,g1); the final 128 slots still cover
                    # {j + 128k}. ScalarE evacuates g4..g7 + g3,g1 + g0lo
                    # (3.5 copies); the DVE folds g2, g0hi from PSUM.
                    # Emission order keeps each copy's source one copy-
                    # duration ahead of ScalarE's queue.
                    mm(4, 0)
                    mm(5, 1024)
                    nc.scalar.copy(s_t[:, 0:2048], P[:, 0:2048])
                    mm(6, 2048)
                    mm(7, 3072)
                    nc.scalar.copy(s_t[:, 2048:4096], P[:, 2048:4096])
                    nc.vector.tensor_tensor(                 # q1 = (s4,s5)
                        q1[:], s_t[:, 0:1024], s_t[:, 1024:2048], op=MAX)
                    nc.vector.tensor_tensor(                 # q2 = (s6,s7)
                        q2[:], s_t[:, 2048:3072], s_t[:, 3072:4096], op=MAX)
                    mm(3, 0)
                    mm(1, 1024)
                    # s3, s1 split so S0 frees one copy early: PE can
                    # refill next tile's g4 during the s1 copy.
                    nc.scalar.copy(s_t[:, 4096:5120], P[:, 0:1024])
                    nc.scalar.copy(s_t[:, 5120:6144], P[:, 1024:2048])
                    nc.vector.tensor_tensor(r1[:], q1[:], q2[:], op=MAX)
                    mm(2, 2048)
                    nc.vector.tensor_tensor(                 # v1 = (g2, s3)
                        v1[:], P[:, 2048:3072], s_t[:, 4096:5120], op=MAX)
                    mm(0, 3072)
                    nc.scalar.copy(s_t[:, 6144:6656], P[:, 3072:3584])
                    nc.vector.tensor_tensor(                 # v2hi = (g0hi, s1hi)
                        v2[:, 512:1024], P[:, 3584:4096],
                        s_t[:, 5632:6144], op=MAX)
                    nc.vector.tensor_tensor(                 # v2lo = (s0lo, s1lo)
                        v2[:, 0:512], s_t[:, 6144:6656],
                        s_t[:, 5120:5632], op=MAX)
                    nc.vector.tensor_tensor(r2[:], v1[:], v2[:], op=MAX)
                    nc.vector.tensor_tensor(ff[:], r1[:], r2[:], op=MAX)
                    nc.vector.tensor_tensor(
                        f3[:], ff[:, 0:512], ff[:, 512:1024], op=MAX)
                    nc.vector.tensor_tensor(
                        f4[:], f3[:, 0:256], f3[:, 256:512], op=MAX)
                    nc.vector.tensor_tensor(
                        f5[:], f4[:, 0:128], f4[:, 128:256], op=MAX)
                    ob = (p * NT + t) * 8
                    nc.vector.max(m8[:], f5[:])
                    nc.vector.max_index(outi_t[:, ob:ob + 8], m8[:], f5[:])
                osl = slice(p * NT * 8, (p + 1) * NT * 8)
                nc.sync.dma_start(outi_d[:, osl], outi_t[:, osl])
    nc.compile()
    return nc


_NC = None
LAST_RESULTS = None  # most recent BassKernelResults (for profiling harnesses)


def _get_nc():
    global _NC
    if _NC is None:
        _NC = _build_nc()
    return _NC


def _prep_inputs(xyz1, xyz2):
    """Build per-core in_maps."""
    xyz1 = np.asarray(xyz1, F32)
    xyz2 = np.asarray(xyz2, F32)
    sq1 = (xyz1.astype(np.float64) ** 2).sum(-1).astype(F32)  # [B,N]
    sq2 = (xyz2.astype(np.float64) ** 2).sum(-1).astype(F32)  # [B,M]

    combs = [np.empty((K, NPROB * PROBW), BF16) for _ in range(NCORES)]
    for b in range(B):
        for rev in (0, 1):
            p = 2 * rev + b
            qsrc = xyz2[b] if rev else xyz1[b]
            dbsrc = xyz1[b] if rev else xyz2[b]
            nsqdb = -(sq1[b] if rev else sq2[b])
            nsqq = -(sq2[b] if rev else sq1[b])
            lhsT_full, rhs = _pack_terms((2.0 * qsrc).astype(F32), dbsrc,
                                         nsqdb, nsqq)
            for c in range(NCORES):
                sl = slice(p * PROBW, p * PROBW + SLAB)
                combs[c][:, sl] = lhsT_full[:, c * SLAB:(c + 1) * SLAB]
                combs[c][:, p * PROBW + SLAB:(p + 1) * PROBW] = rhs
    return [{"comb": combs[c]} for c in range(NCORES)]


def kernel(xyz1, xyz2):
    xyz1 = np.asarray(xyz1, F32)
    xyz2 = np.asarray(xyz2, F32)
    nc = _get_nc()
    in_maps = _prep_inputs(xyz1, xyz2)
    global LAST_RESULTS
    LAST_RESULTS = run_bass_kernel_spmd(nc, in_maps, list(range(NCORES)))
    res = LAST_RESULTS.results

    # assemble argmax slots: slots[p][global query row] in [0, 256)
    slots = np.empty((NPROB, N), np.int64)
    for c in range(NCORES):
        outi = np.asarray(res[c]["outi"]).reshape(TQ, NPROB * NT * 8)
        for p in range(NPROB):
            for t in range(NT):
                rows = slice(c * SLAB + t * TQ, c * SLAB + (t + 1) * TQ)
                slots[p, rows] = outi[:, (p * NT + t) * 8].astype(np.int64)

    dist1 = np.empty((B, N), F32)
    dist2 = np.empty((B, M), F32)
    idx1 = np.empty((B, N), np.int32)
    idx2 = np.empty((B, M), np.int32)
    offs = NSLOT * np.arange(NCAND)[None, :]        # [1, 32]
    for b in range(B):
        for rev in (0, 1):
            p = 2 * rev + b
            q = (xyz2[b] if rev else xyz1[b]).astype(np.float64)
            db = (xyz1[b] if rev else xyz2[b]).astype(np.float64)
            cand = slots[p][:, None] + offs                  # [N, 32]
            dd = ((q[:, None, :] - db[cand]) ** 2).sum(-1)   # [N, 32] fp64
            best = dd.argmin(1)
            rows = np.arange(N)
            if rev:
                dist2[b] = dd[rows, best].astype(F32)
                idx2[b] = cand[rows, best].astype(np.int32)
            else:
                dist1[b] = dd[rows, best].astype(F32)
                idx1[b] = cand[rows, best].astype(np.int32)
    return dist1, dist2, idx1, idx2


# revision 34
# speedup vs baseline: 1.0412x; 1.0039x over previous
"""Chamfer distance kernel for 8x Trainium2 NeuronCores (Bass/Tile).

Problem: xyz1 [2,8192,3] f32, xyz2 [2,8192,3] f32 ->
  dist1 [2,8192] f32, dist2 [2,8192] f32, idx1 [2,8192] i32, idx2 [2,8192] i32
  (squared L2 nearest-neighbor distances + argmins, both directions).

Sharding: core c owns rows c*1024:(c+1)*1024 of xyz1 (forward direction,
min over full xyz2) AND rows c*1024:(c+1)*1024 of xyz2 (reverse direction,
min over full xyz1). Each core's outputs are exact output shards -> host
just concatenates (no collectives).

Math: the device computes, per query q and db point x_j,
  e[q,j] = 2 q.x_j - |x_j|^2 - |q|^2  (= -|q - x_j|^2, so argmax_j e =
argmin_j d and |e| is SMALL near the max -> fp16 keeps ~2^-11 relative
precision exactly where the argmax is decided). e is produced by ONE bf16
matmul with K=24 packed rows: each fp32 operand split into bf16 limbs
(h/m/l), all cross terms down to ~2^-27 relative; -|x|^2 and -|q|^2 are
folded in via constant-one rows.

Per 128-query row-tile the device then:
  - 16 matmuls [128,512] into a manually-scheduled [128,4096] PSUM region
    (8 groups of 1024 cols, two rounds over the region; slice-level tile
    deps give a 4-deep rotation)
  - evacuates PSUM: ScalarE copies 6.5 groups to SBUF as fp16 (three
    2048-wide copies + one 512) while the DVE max-folds the remaining 1.5
    groups directly from PSUM (evacuation + first fold level in one pass).
    GPSIMD cannot help: the Pool engine has no max ALU op on TRN2.
  - binary max-fold tree (pairs chosen within PSUM rounds so every copy's
    source is written ahead of ScalarE's queue) down to f5 [128,128],
    all on the DVE at fp16 2x_1P
  - InstMax + InstMaxIndex on f5 give the argmax SLOT (8 deep)
The host expands slot -> 64 column candidates (stride 128), evaluates the
true distances in fp64, and takes the min: exact dist at the chosen
index, index correct up to fp16 near-ties (same class of ties the fp32
reference resolves arbitrarily itself).
"""

import numpy as np
import ml_dtypes

import concourse.bacc as bacc
import concourse.mybir as mybir
import concourse.tile as tile
from concourse.bass_utils import run_bass_kernel_spmd

BF16 = ml_dtypes.bfloat16
F32 = np.float32

NCORES = 8
B, N, M, C = 2, 8192, 8192, 3
SLAB = N // NCORES            # 1024 queries per core per problem
NPROB = 2 * B                 # (fwd,b0),(fwd,b1),(rev,b0),(rev,b1)
K = 24                        # packed contraction rows
TQ = 128                      # queries per row-tile (partitions)
NT = SLAB // TQ               # 8 row-tiles per problem
MCH = 512                     # matmul free chunk (one PSUM bank)
PSW = 2048                    # psum tile width (4 banks; bufs=2 => 8 banks)
NG = M // PSW                 # 4 psum groups per row-tile
NSLOT = 128                   # folded slot count (f6 width)
NCAND = M // NSLOT            # 64 candidates per slot
PROBW = SLAB + M              # per-problem packed operand width


def _split3(x):
    """fp32 -> three bf16 limbs (as fp32 arrays) with x ~= h+m+l to ~2^-27."""
    x = x.astype(F32)
    h = x.astype(BF16)
    r = (x - h.astype(F32)).astype(F32)
    m = r.astype(BF16)
    r2 = (r - m.astype(F32)).astype(F32)
    l = r2.astype(BF16)
    return h.astype(F32), m.astype(F32), l.astype(F32)


def _pack_terms(q2, db, neg_sqdb, neg_sqq):
    """Build the K=24 (lhs_col, rhs_row) packed operands.

    q2: [Nq,3] fp32 (queries, ALREADY scaled by 2)
    db: [M,3] fp32, neg_sqdb: [M] fp32 (= -|db|^2), neg_sqq: [Nq] (= -|q|^2)
    Returns lhsT [K,Nq] bf16, rhs [K,M] bf16. Big terms first so PSUM
    partial sums stay well-scaled.
    """
    h1, m1, l1 = _split3(q2)
    h2, m2, l2 = _split3(db)
    sh, sm, sl = _split3(neg_sqdb)
    qh, qm, ql = _split3(neg_sqq)
    ones_n = np.ones(q2.shape[0], F32)
    ones_m = np.ones(db.shape[0], F32)
    lhs_rows, rhs_rows = [], []

    def add(lc, rr):
        lhs_rows.append(lc)
        rhs_rows.append(rr)

    for c in range(3):
        add(h1[:, c], h2[:, c])
    add(ones_n, sh)
    add(qh, ones_m)
    for c in range(3):
        add(h1[:, c], m2[:, c])
        add(m1[:, c], h2[:, c])
    add(ones_n, sm)
    add(qm, ones_m)
    for c in range(3):
        add(m1[:, c], m2[:, c])
        add(h1[:, c], l2[:, c])
        add(l1[:, c], h2[:, c])
    add(ones_n, sl)
    add(ql, ones_m)
    assert len(lhs_rows) == K
    lhsT = np.stack(lhs_rows, 0).astype(BF16)
    rhs = np.stack(rhs_rows, 0).astype(BF16)
    return lhsT, rhs


def _build_nc():
    # Bacc (not plain Bass): its compile() pipeline moves matmul waits onto
    # ldweights and splits multi-wait instructions via event semaphores.
    nc = bacc.Bacc("TRN2", target_bir_lowering=False, debug=False)
    comb_d = nc.dram_tensor("comb", [K, NPROB * PROBW], mybir.dt.bfloat16,
                            kind="ExternalInput")
    # argmax slot per (problem, tile): 8 MaxIndex outputs, col 0 is the slot.
    outi_d = nc.dram_tensor("outi", [TQ, NPROB * NT * 8], mybir.dt.uint16,
                            kind="ExternalOutput")

    F16 = mybir.dt.float16
    MAX = mybir.AluOpType.max

    with tile.TileContext(nc) as tc:
        with (
            tc.tile_pool(name="const", bufs=1) as constp,
            tc.tile_pool(name="ebuf", bufs=3) as ep,
            tc.tile_pool(name="psum", bufs=1, space="PSUM") as pp,
        ):
            comb_t = constp.tile([K, NPROB * PROBW], mybir.dt.bfloat16)
            # DMA order follows first-use order inside problem 0 (queries,
            # then rhs groups g4g5, g6g7, g3, g1, g2, g0) so compute starts
            # asap; remaining problems stream behind compute.
            for a, b_ in ((0, SLAB), (SLAB + 4096, SLAB + 5120),
                          (SLAB + 5120, SLAB + 6144),
                          (SLAB + 6144, PROBW), (SLAB + 3072, SLAB + 4096),
                          (SLAB + 1024, SLAB + 2048),
                          (SLAB + 2048, SLAB + 3072), (SLAB, SLAB + 1024)):
                nc.sync.dma_start(comb_t[:, a:b_], comb_d[:, a:b_])
            for p in range(1, NPROB):
                sl_p = slice(p * PROBW, (p + 1) * PROBW)
                nc.sync.dma_start(comb_t[:, sl_p], comb_d[:, sl_p])
            outi_t = constp.tile([TQ, NPROB * NT * 8], mybir.dt.uint16)
            # hoist the ScalarE activation-table load (~2.7us) into the
            # initial DMA window: prime it with a tiny copy on scratch.
            atl_t = constp.tile([TQ, 8], mybir.dt.float32)
            nc.vector.memset(atl_t[:], 0.0)
            nc.scalar.copy(atl_t[:], atl_t[:])

            # one manually-scheduled PSUM region [TQ, 4096] (all 8 banks);
            # slice-level dep tracking gives 4-deep 1024-col rotation while
            # letting ScalarE evacuate 2048-col spans in one instruction.
            P = pp.tile([TQ, 4096], mybir.dt.float32)

            for p in range(NPROB):
                base = p * PROBW
                for t in range(NT):
                    lhs = comb_t[:, base + t * TQ:base + (t + 1) * TQ]
                    # s_t: [s4 | s5 | s6 | s7 | s3 | s1 | s0lo(512)] (fp16)
                    s_t = ep.tile([TQ, 6 * 1024 + 512], F16, tag="s")
                    q1 = ep.tile([TQ, 1024], F16, tag="q1")
                    q2 = ep.tile([TQ, 1024], F16, tag="q2")
                    v1 = ep.tile([TQ, 1024], F16, tag="v1")
                    v2 = ep.tile([TQ, 1024], F16, tag="v2")
                    r1 = ep.tile([TQ, 1024], F16, tag="r1")
                    r2 = ep.tile([TQ, 1024], F16, tag="r2")
                    ff = ep.tile([TQ, 1024], F16, tag="ff")
                    f3 = ep.tile([TQ, 512], F16, tag="f3")
                    f4 = ep.tile([TQ, 256], F16, tag="f4")
                    f5 = ep.tile([TQ, NSLOT], F16, tag="f5")
                    m8 = ep.tile([TQ, 8], F16, tag="m8")

                    def mm(g, pcol):
                        # group g (db cols [g*1024,(g+1)*1024)) -> P[:, pcol:+1024]
                        o = base + SLAB + g * 1024
                        for j in range(1024 // MCH):
                            nc.tensor.matmul(
                                P[:, pcol + j * MCH:pcol + (j + 1) * MCH],
                                lhs,
                                comb_t[:, o + j * MCH:o + (j + 1) * MCH],
                                start=True, stop=True,
                            )

                    # 1024-col groups g0..g7 paired within PSUM rounds:
                    # q1=max(g4,g5), q2=max(g6,g7), v1=max(g2,g3),
                    # v2=max(g0,g1); the final 128 slots still cover
                    # {j + 128k}. ScalarE evacuates g4..g7 + g3,g1 + g0lo
                    # (3.5 copies); the DVE folds g2, g0hi from PSUM.
                    # Emission order keeps each copy's source one copy-
                    # duration ahead of ScalarE's queue.
                    mm(4, 0)
                    mm(5, 1024)
                    nc.scalar.copy(s_t[:, 0:2048], P[:, 0:2048])
                    mm(6, 2048)
                    mm(7, 3072)
                    nc.scalar.copy(s_t[:, 2048:3840], P[:, 2048:3840])
                    nc.vector.tensor_tensor(                 # q1 = (s4,s5)
                        q1[:], s_t[:, 0:1024], s_t[:, 1024:2048], op=MAX)
                    nc.vector.tensor_tensor(                 # q2hi = (g7hi-PSUM, s6hi)
                        q2[:, 768:1024], P[:, 3840:4096],
                        s_t[:, 2816:3072], op=MAX)
                    nc.vector.tensor_tensor(                 # q2lo = (s6lo,s7lo)
                        q2[:, 0:768], s_t[:, 2048:2816],
                        s_t[:, 3072:3840], op=MAX)
                    mm(3, 0)
                    mm(1, 1024)
                    # s3, s1 split so S0 frees one copy early: PE can
                    # refill next tile's g4 during the s1 copy.
                    nc.scalar.copy(s_t[:, 4096:5120], P[:, 0:1024])
                    nc.scalar.copy(s_t[:, 5120:6144], P[:, 1024:2048])
                    nc.vector.tensor_tensor(r1[:], q1[:], q2[:], op=MAX)
                    mm(2, 2048)
                    nc.vector.tensor_tensor(                 # v1 = (g2, s3)
                        v1[:], P[:, 2048:3072], s_t[:, 4096:5120], op=MAX)
                    mm(0, 3072)
                    nc.scalar.copy(s_t[:, 6144:6656], P[:, 3072:3584])
                    nc.vector.tensor_tensor(                 # v2hi = (g0hi, s1hi)
                        v2[:, 512:1024], P[:, 3584:4096],
                        s_t[:, 5632:6144], op=MAX)
                    nc.vector.tensor_tensor(                 # v2lo = (s0lo, s1lo)
                        v2[:, 0:512], s_t[:, 6144:6656],
                        s_t[:, 5120:5632], op=MAX)
                    nc.vector.tensor_tensor(r2[:], v1[:], v2[:], op=MAX)
                    nc.vector.tensor_tensor(ff[:], r1[:], r2[:], op=MAX)
                    nc.vector.tensor_tensor(
                        f3[:], ff[:, 0:512], ff[:, 512:1024], op=MAX)
                    nc.vector.tensor_tensor(
                        f4[:], f3[:, 0:256], f3[:, 256:512], op=MAX)
                    nc.vector.tensor_tensor(
                        f5[:], f4[:, 0:128], f4[:, 128:256], op=MAX)
                    ob = (p * NT + t) * 8
                    nc.vector.max(m8[:], f5[:])
                    nc.vector.max_index(outi_t[:, ob:ob + 8], m8[:], f5[:])
                osl = slice(p * NT * 8, (p + 1) * NT * 8)
                nc.sync.dma_start(outi_d[:, osl], outi_t[:, osl])
    nc.compile()
    return nc


_NC = None
LAST_RESULTS = None  # most recent BassKernelResults (for profiling harnesses)


def _get_nc():
    global _NC
    if _NC is None:
        _NC = _build_nc()
    return _NC


def _prep_inputs(xyz1, xyz2):
    """Build per-core in_maps."""
    xyz1 = np.asarray(xyz1, F32)
    xyz2 = np.asarray(xyz2, F32)
    sq1 = (xyz1.astype(np.float64) ** 2).sum(-1).astype(F32)  # [B,N]
    sq2 = (xyz2.astype(np.float64) ** 2).sum(-1).astype(F32)  # [B,M]

    combs = [np.empty((K, NPROB * PROBW), BF16) for _ in range(NCORES)]
    for b in range(B):
        for rev in (0, 1):
            p = 2 * rev + b
            qsrc = xyz2[b] if rev else xyz1[b]
            dbsrc = xyz1[b] if rev else xyz2[b]
            nsqdb = -(sq1[b] if rev else sq2[b])
            nsqq = -(sq2[b] if rev else sq1[b])
            lhsT_full, rhs = _pack_terms((2.0 * qsrc).astype(F32), dbsrc,
                                         nsqdb, nsqq)
            for c in range(NCORES):
                sl = slice(p * PROBW, p * PROBW + SLAB)
                combs[c][:, sl] = lhsT_full[:, c * SLAB:(c + 1) * SLAB]
                combs[c][:, p * PROBW + SLAB:(p + 1) * PROBW] = rhs
    return [{"comb": combs[c]} for c in range(NCORES)]


def kernel(xyz1, xyz2):
    xyz1 = np.asarray(xyz1, F32)
    xyz2 = np.asarray(xyz2, F32)
    nc = _get_nc()
    in_maps = _prep_inputs(xyz1, xyz2)
    global LAST_RESULTS
    LAST_RESULTS = run_bass_kernel_spmd(nc, in_maps, list(range(NCORES)))
    res = LAST_RESULTS.results

    # assemble argmax slots: slots[p][global query row] in [0, 256)
    slots = np.empty((NPROB, N), np.int64)
    for c in range(NCORES):
        outi = np.asarray(res[c]["outi"]).reshape(TQ, NPROB * NT * 8)
        for p in range(NPROB):
            for t in range(NT):
                rows = slice(c * SLAB + t * TQ, c * SLAB + (t + 1) * TQ)
                slots[p, rows] = outi[:, (p * NT + t) * 8].astype(np.int64)

    dist1 = np.empty((B, N), F32)
    dist2 = np.empty((B, M), F32)
    idx1 = np.empty((B, N), np.int32)
    idx2 = np.empty((B, M), np.int32)
    offs = NSLOT * np.arange(NCAND)[None, :]        # [1, 32]
    for b in range(B):
        for rev in (0, 1):
            p = 2 * rev + b
            q = (xyz2[b] if rev else xyz1[b]).astype(np.float64)
            db = (xyz1[b] if rev else xyz2[b]).astype(np.float64)
            cand = slots[p][:, None] + offs                  # [N, 32]
            dd = ((q[:, None, :] - db[cand]) ** 2).sum(-1)   # [N, 32] fp64
            best = dd.argmin(1)
            rows = np.arange(N)
            if rev:
                dist2[b] = dd[rows, best].astype(F32)
                idx2[b] = cand[rows, best].astype(np.int32)
            else:
                dist1[b] = dd[rows, best].astype(F32)
                idx1[b] = cand[rows, best].astype(np.int32)
    return dist1, dist2, idx1, idx2
